# revision 33
# baseline (speedup 1.0000x reference)
"""Trainium2 Bass kernel for nn_Attention (B=4, C=256, L=2048, H=8 heads, D=64).

Sharding: head-parallel across 8 NeuronCores (1 head per core). Each core:
  - projects its head's Q/K/V from the full input x (channels-first, fp16),
  - runs attention in the S^T (keys-on-partitions) layout so softmax's
    denominator comes free from an appended ones-column in the V^T lhsT
    (M=65 matmul),
  - normalizes + casts its head output to fp16,
  - per half-batch AllToAlls redistribute head outputs: each core k owns
    64-column granules g with g % 8 == k of every batch (granule mapping:
    l = ib*512 + k*64 + t for shard column s = ib*64 + t),
  - each core applies w_out + bias on its column shard.
Host reassembles the 8 granule-interleaved shards into the full [B, C, L]
output.

The attention phase is ACT(Exp)-bound (~1.1us per (ib,jp) step); the whole
(b, ib, jp) sequence is software-pipelined as one stream with the PV matmul
running two steps behind S, and the PE's leftover per-step time is filled
with the next batch's QKV projection and the previous batch's output
projection, one small chunk per slot, so the Exp stream never starves.
"""

import os
import sys
from collections import deque

import numpy as np

sys.path.insert(0, "/opt/trn_rl_repo")

import concourse.bass as bass  # noqa: E402
import concourse.bacc as bacc  # noqa: E402
import concourse.tile as tile  # noqa: E402
import concourse.mybir as mybir  # noqa: E402
import concourse.bass_utils as bass_utils  # noqa: E402
from concourse.bass_interp import get_hw_module  # noqa: E402

B, C, L = 4, 256, 2048
H, D = 8, 64
NCORES = 8
N = B * L                # 8192 flattened (b, l) columns
LSH = L // NCORES        # 256 l-columns per core in the output shard
NBLK = 512               # matmul free-dim block
F32 = mybir.dt.float32
F16 = mybir.dt.float16
AF = mybir.ActivationFunctionType

_CACHE = {}


def _kernel_hash():
    import hashlib
    with open(os.path.abspath(__file__), 'rb') as f:
        return int(hashlib.sha256(f.read()).hexdigest(), 16)


def _build():
    nc = bacc.Bacc("TRN2", target_bir_lowering=False, debug=False,
                   num_devices=NCORES)

    # The executable cache upstream keys on the HLO signature, which does not
    # cover the embedded BIR — encode a hash of this file in a dummy input's
    # shape so different kernel builds can't collide.
    nonce_len = _kernel_hash() % 509 + 3
    nc.dram_tensor("nonce", [1, nonce_len], F32, kind="ExternalInput")

    x_t = nc.dram_tensor("x_t", [2, 128, N], F16, kind="ExternalInput")
    # [c_lo, ch, (q|k) out] merged Q+K projection weights
    wqk_p = nc.dram_tensor("wqk_p", [128, 2, 128], F16, kind="ExternalInput")
    wv_p = nc.dram_tensor("wv_p", [128, 128], F16, kind="ExternalInput")
    wo_p = nc.dram_tensor("wo_p", [128, 4, 256], F16, kind="ExternalInput")
    bias2 = nc.dram_tensor("bias2", [128, 2], F32, kind="ExternalInput")
    out = nc.dram_tensor("out", [B, 2, 128, LSH], F32, kind="ExternalOutput")

    with tile.TileContext(nc) as tc:
        with (
            tc.tile_pool(name="const", bufs=1) as cpool,
            tc.tile_pool(name="qk", bufs=3) as qkpool,
            tc.tile_pool(name="vt", bufs=3) as vtpool,
            tc.tile_pool(name="pt", bufs=4) as ptpool,
            tc.tile_pool(name="small", bufs=4) as spool,
            tc.tile_pool(name="gh", bufs=2) as ghpool,
            tc.tile_pool(name="psA", bufs=2, space="PSUM") as psA,
            tc.tile_pool(name="psO", bufs=2, space="PSUM") as psO,
            tc.tile_pool(name="psP", bufs=2, space="PSUM") as psP,
            tc.tile_pool(name="dram", bufs=1, space="DRAM") as dpool,
        ):
            # ---- weights + batch-0 x first (fine chunks so the batch-0
            # projection can start as soon as each 512-col block lands) ----
            wqk_sb = cpool.tile([128, 256], F16, name="wqk_sb")
            wv_sb = cpool.tile([128, 128], F16, name="wv_sb")
            wo_sb = cpool.tile([128, 1024], F16, name="wo_sb")
            bias_sb = cpool.tile([128, 2], F32, name="bias_sb")
            nc.sync.dma_start(wqk_sb.rearrange("p (c o) -> p c o", c=2), wqk_p[:])
            nc.sync.dma_start(wv_sb[:], wv_p[:])
            x_sb = cpool.tile([128, 2 * N], F16, name="x_sb")

            # tiny warmup AllToAll: absorbs communicator init + first-use
            # slowness while the x DMA streams, so the first real a2a runs
            # at steady-state speed
            warm_sb = cpool.tile([NCORES, 2], F16, name="warm_sb")
            nc.vector.memset(warm_sb[:], 0.0)
            warm_in = dpool.tile([NCORES, 1, 2], F16, name="warm_in",
                                 tag="warm_in")
            warm_out = dpool.tile([NCORES, 1, 2], F16, name="warm_out",
                                  tag="warm_out")
            nc.sync.dma_start(warm_in[:, 0, :], warm_sb[:])
            nc.gpsimd.collective_compute(
                "AllToAll", mybir.AluOpType.bypass,
                replica_groups=[list(range(NCORES))],
                ins=[warm_in.opt()], outs=[warm_out.opt()])

            def dma_x(c0, c1):
                for ch in range(2):
                    nc.sync.dma_start(
                        x_sb[:, ch * N + c0:ch * N + c1],
                        x_t[ch, :, c0:c1])

            for blk in range(4):                     # batch 0, 512-col blocks
                dma_x(blk * NBLK, (blk + 1) * NBLK)
            nc.sync.dma_start(wo_sb.rearrange("p (c o) -> p c o", c=4), wo_p[:])
            nc.sync.dma_start(bias_sb[:], bias2[:])
            for s in range(2, 8):                    # batches 1-3
                dma_x(s * 1024, (s + 1) * 1024)

            # a2a staging: batches 0-2 use one whole-batch AllToAll;
            # the last batch is split into two half-batch AllToAlls (with
            # separate tiles so the Tile dependency tracker can't couple the
            # halves) to shrink the end-of-kernel tail.
            def _mk_bnc(pfx):
                ts = [dpool.tile([NCORES, 64, 256], F16, name=f"{pfx}{b}",
                                 tag=f"{pfx}{b}") for b in range(B - 1)]
                last = [dpool.tile([NCORES, 64, 192], F16, name=f"{pfx}30",
                                   tag=f"{pfx}30"),
                        dpool.tile([NCORES, 64, 64], F16, name=f"{pfx}31",
                                   tag=f"{pfx}31")]
                return ts, last
            bnc_in, bnc_in3 = _mk_bnc("bi")
            bnc_out, bnc_out3 = _mk_bnc("bo")

            qd = {}
            kd = {}
            vt3 = {}

            # ---------------- filler emission (PE keep-warm) ----------------
            # Each filler is tagged 'pe' (matmul work) or 'vec' (vector/DMA
            # work). 'vec' fillers are only popped in mid-ib slots so they
            # can't queue ahead of the per-ib recip/mul normalization chain
            # on the vector engine (which feeds the a2a trigger).
            fillers = deque()

            def filler(vec_ok=True):
                # pop the first item allowed in this slot whose python-level
                # prerequisites have been emitted (scan a small window so a
                # blocked head can't starve the slot)
                for idx in range(min(len(fillers), 8)):
                    kind, ready, fn = fillers[idx][:3]
                    if (vec_ok or kind == 'pe') and (ready is None or ready()):
                        del fillers[idx]
                        fn()
                        return

            def drain_tagged(tag):
                # Tile tracks data deps in emission order, so everything a
                # batch consumes must be emitted before its first consumer:
                # force-drain the tagged package at its deadline.
                rest = deque()
                while fillers:
                    item = fillers.popleft()
                    if item[3] == tag:
                        item[2]()
                    else:
                        rest.append(item)
                fillers.extend(rest)

            def drain_fillers():
                while fillers:
                    fillers.popleft()[2]()

            def emit_projvt_blk(b, blk, st):
                """QK proj + V^T for one 512-col block of batch b (direct
                emission; used for batch 0 under the initial x DMA)."""
                emit_qk_mm(b, blk, 0, st)
                emit_qk_mm(b, blk, 1, st)
                emit_qk_copy(b, blk, st)
                for jp in range(2 * blk, 2 * blk + 2):
                    emit_vt_mm(b, jp, 0, st)
                    emit_vt_mm(b, jp, 1, st)
                    emit_vt_copy(b, jp, st)

            def emit_alloc(b):
                qd[b] = qkpool.tile([128, L], F16, name="qd", tag="qd")
                kd[b] = qkpool.tile([128, L], F16, name="kd", tag="kd")
                vt3[b] = vtpool.tile(
                    [128, 16 * 65], F16, name="vt", tag="vt"
                ).rearrange("p (j e) -> p j e", e=65)
                nc.vector.memset(vt3[b][:, :, 64], 1.0)

            def emit_qk_mm(b, nb, ch, st):
                if ch == 0:
                    st[('ps', nb)] = psP.tile([128, NBLK], F32, name="psqk",
                                              tag="psp")
                col0 = ch * N + b * L + nb * NBLK
                nc.tensor.matmul(
                    st[('ps', nb)][:], wqk_sb[:, ch * 128:(ch + 1) * 128],
                    x_sb[:, col0:col0 + NBLK],
                    start=(ch == 0), stop=(ch == 1))

            def emit_qk_copy(b, nb, st):
                ps = st.pop(('ps', nb))
                nc.vector.tensor_copy(
                    qd[b][0:64, nb * NBLK:(nb + 1) * NBLK], ps[0:64, :])
                nc.vector.tensor_copy(
                    kd[b][0:64, nb * NBLK:(nb + 1) * NBLK], ps[64:128, :])
                nc.vector.tensor_copy(
                    qd[b][64:128, nb * NBLK:(nb + 1) * NBLK],
                    qd[b][0:64, nb * NBLK:(nb + 1) * NBLK])
                nc.vector.tensor_copy(
                    kd[b][64:128, nb * NBLK:(nb + 1) * NBLK],
                    kd[b][0:64, nb * NBLK:(nb + 1) * NBLK])

            def emit_vt_mm(b, jp, half, st):
                # V^T directly: out[l, d] = sum_c x[c, l] * wv[c, d]
                if half == 0:
                    st[('pst', jp)] = psP.tile([128, 128], F32, name="pst",
                                               tag="psp")
                jt = 2 * jp + half
                for ch in range(2):
                    col0 = ch * N + b * L + jt * 128
                    nc.tensor.matmul(
                        st[('pst', jp)][:, half * 64:(half + 1) * 64],
                        x_sb[:, col0:col0 + 128],
                        wv_sb[:, ch * 64:(ch + 1) * 64],
                        start=(ch == 0), stop=(ch == 1))

            def emit_vt_copy(b, jp, st):
                nc.vector.tensor_copy(
                    vt3[b][:, 2 * jp:2 * jp + 2, 0:64],
                    st.pop(('pst', jp)).rearrange("p (j e) -> p j e", e=64))

            def queue_projvt(b):
                st = {}
                emit_alloc(b)
                tag = ('proj', b)
                for nb in range(4):
                    fillers.append(('pe', None,
                                    lambda nb=nb: emit_qk_mm(b, nb, 0, st), tag))
                    fillers.append(('pe', None,
                                    lambda nb=nb: emit_qk_mm(b, nb, 1, st), tag))
                    fillers.append(('vec',
                                    (lambda nb=nb: ('ps', nb) in st),
                                    lambda nb=nb: emit_qk_copy(b, nb, st), tag))
                for jp in range(8):
                    fillers.append(('pe', None,
                                    lambda jp=jp: emit_vt_mm(b, jp, 0, st), tag))
                    fillers.append(('pe', None,
                                    lambda jp=jp: emit_vt_mm(b, jp, 1, st), tag))
                    fillers.append(('vec',
                                    (lambda jp=jp: ('pst', jp) in st),
                                    lambda jp=jp: emit_vt_copy(b, jp, st), tag))

            def queue_yproj(b, h):
                """Output projection of batch b, piece h (after the (b, h)
                a2a). Batches 0-2 split 128+128 shard columns; batch 3
                splits 192+64 to match its asymmetric a2as."""
                if b == B - 1:
                    c0, c1 = (0, 192) if h == 0 else (192, 256)
                else:
                    c0, c1 = (128 * h, 128 * h + 128)
                w = c1 - c0
                st = {}

                def gather():
                    st['gh'] = ghpool.tile([128, 4, w], F16, name="gh",
                                           tag=f"gh{h}")
                    for hc in range(4):
                        if b == B - 1:
                            src_ap = bnc_out3[h][hc * 2:hc * 2 + 2, :, :]
                        else:
                            src_ap = bnc_out[b][hc * 2:hc * 2 + 2, :, c0:c1]
                        nc.sync.dma_start(
                            st['gh'][:, hc, :],
                            src_ap.rearrange("w p t -> (w p) t"))

                tag = ('yproj', b, h)
                fillers.append(('vec', None, gather, tag))

                def y_mm(oh, cpair):
                    if cpair == 0:
                        st[('psy', oh)] = psP.tile([128, w], F32, name="psy",
                                                   tag="psp")
                    for c in (2 * cpair, 2 * cpair + 1):
                        nc.tensor.matmul(
                            st[('psy', oh)][:],
                            wo_sb[:, c * 256 + oh * 128:c * 256 + (oh + 1) * 128],
                            st['gh'][:, c, :],
                            start=(c == 0), stop=(c == 3))

                def y_out(oh):
                    y = spool.tile([128, w], F32, name="y", tag="y")
                    nc.vector.tensor_scalar_add(y[:], st.pop(('psy', oh)),
                                                bias_sb[:, oh:oh + 1])
                    nc.sync.dma_start(out[b, oh, :, c0:c1], y[:])

                for oh in range(2):
                    fillers.append(('pe', (lambda: 'gh' in st),
                                    lambda oh=oh: y_mm(oh, 0), tag))
                    fillers.append(('pe', (lambda oh=oh: ('psy', oh) in st),
                                    lambda oh=oh: y_mm(oh, 1), tag))
                    fillers.append(('vec', (lambda oh=oh: ('psy', oh) in st),
                                    lambda oh=oh: y_out(oh), tag))

            # ---------------- attention stream ----------------
            psos = {}
            pts = {}

            def emit_s(b, ib, jp):
                jA, jB = 2 * jp, 2 * jp + 1
                pss = psA.tile([128, 2 * NBLK], F32, name="pss", tag="pss")
                nc.tensor.matmul(
                    pss[:, 0:NBLK],
                    kd[b][0:64, jA * 128:(jA + 1) * 128],
                    qd[b][0:64, ib * NBLK:(ib + 1) * NBLK],
                    start=True, stop=True, tile_position=(0, 0))
                nc.tensor.matmul(
                    pss[:, NBLK:2 * NBLK],
                    kd[b][64:128, jB * 128:(jB + 1) * 128],
                    qd[b][64:128, ib * NBLK:(ib + 1) * NBLK],
                    start=True, stop=True, tile_position=(64, 0))
                pt = ptpool.tile([128, 2 * NBLK], F16, name="pt", tag="pt")
                nc.scalar.activation(pt[:], pss[:], AF.Exp)
                pts[(b, ib, jp)] = pt

            def emit_pv(b, ib, jp):
                jA, jB = 2 * jp, 2 * jp + 1
                if jp == 0:
                    psos[(b, ib)] = psO.tile([65, NBLK], F32, name="pso",
                                             tag="pso")
                pso = psos[(b, ib)]
                pt = pts.pop((b, ib, jp))
                nc.tensor.matmul(
                    pso[:], vt3[b][:, jA, :], pt[:, 0:NBLK],
                    start=(jp == 0), stop=False)
                nc.tensor.matmul(
                    pso[:], vt3[b][:, jB, :], pt[:, NBLK:2 * NBLK],
                    start=False, stop=(jp == 7))
                if jp == 7:
                    emit_norm_out(b, ib)

            def emit_norm_out(b, ib):
                """Normalize + stage ib's output columns for the a2a; then
                trigger the half-batch a2a when its data is complete."""
                pso = psos.pop((b, ib))
                # custom-DVE ops need a base-partition-0 input: stage the
                # denominator row into SBUF first
                den = spool.tile([1, NBLK], F32, name="den", tag="den")
                nc.vector.tensor_copy(den[:], pso[64:65, :])
                recip = spool.tile([1, NBLK], F32, name="recip", tag="recip")
                nc.vector.reciprocal_approx_fast(recip[:], den[:])
                bc = spool.tile([64, NBLK], F32, name="bc", tag="bc")
                nc.gpsimd.partition_broadcast(bc[:], recip[:])
                on = spool.tile([64, NBLK], F16, name="on", tag="on")
                nc.vector.tensor_mul(on[:], pso[0:64, :], bc[:])
                # granule-interleaved shard mapping: column k*64+t of `on`
                # goes to core k's shard slot ib*64+t
                if b == B - 1:
                    if ib < 3:
                        dst = bnc_in3[0][:, :, ib * 64:(ib + 1) * 64]
                    else:
                        dst = bnc_in3[1][:, :, :]
                else:
                    dst = bnc_in[b][:, :, ib * 64:(ib + 1) * 64]
                nc.sync.dma_start(dst.rearrange("k p t -> p k t"),
                                  on.rearrange("p (k t) -> p k t", k=8))
                grp = [list(range(NCORES))]
                if b == B - 1 and ib >= 2:
                    # asymmetric split: the {ib0-2} a2a overlaps ib3's
                    # compute; only a 64-column a2a sits on the tail
                    nc.gpsimd.collective_compute(
                        "AllToAll", mybir.AluOpType.bypass, replica_groups=grp,
                        ins=[bnc_in3[ib - 2].opt()],
                        outs=[bnc_out3[ib - 2].opt()])
                elif b < B - 1 and ib == 3:
                    nc.gpsimd.collective_compute(
                        "AllToAll", mybir.AluOpType.bypass, replica_groups=grp,
                        ins=[bnc_in[b].opt()], outs=[bnc_out[b].opt()])

            # ---------------- schedule ----------------
            # Batch 0's projection is emitted block-by-block just ahead of
            # the attention steps that consume it, so the first Exp can
            # start as soon as the first 512 columns of x have landed.
            st0 = {}
            emit_alloc(0)
            for blk in range(4):
                emit_projvt_blk(0, blk, st0)

            steps = [(b, ib, jp)
                     for b in range(B) for ib in range(4) for jp in range(8)]
            pending = deque()
            for (b, ib, jp) in steps:
                if (ib, jp) == (0, 0) and b > 0:
                    drain_tagged(('proj', b))
                if (ib, jp) == (2, 0):
                    if b + 1 < B:
                        queue_projvt(b + 1)
                    if b > 0:
                        queue_yproj(b - 1, 0)
                if (ib, jp) == (3, 0) and b > 0:
                    queue_yproj(b - 1, 1)
                emit_s(b, ib, jp)
                filler()
                pending.append((b, ib, jp))
                if len(pending) > 2:
                    emit_pv(*pending.popleft())
                    filler()
            while pending:
                emit_pv(*pending.popleft())
            queue_yproj(B - 1, 0)
            queue_yproj(B - 1, 1)
            drain_fillers()

    nc.compile()
    if not os.environ.get("BASS_SIM"):
        nc.m = get_hw_module(nc.m)
    return nc


def _prep_in_maps(x, w_qkv, w_out, b_out):
    scale = float(D) ** -0.5
    x = np.asarray(x, np.float32)
    w_qkv = np.asarray(w_qkv, np.float32)
    w_out = np.asarray(w_out, np.float32)
    b_out = np.asarray(b_out, np.float32)

    x_in = np.ascontiguousarray(
        x.transpose(1, 0, 2).reshape(C, N).reshape(2, 128, N)).astype(np.float16)
    wq = w_qkv[0:512].reshape(H, D, C) * scale
    wk = w_qkv[512:1024].reshape(H, D, C)
    wv = w_qkv[1024:1536].reshape(H, D, C)

    wo_p = np.ascontiguousarray(
        w_out.T.reshape(4, 2, 64, 256).transpose(1, 2, 0, 3).reshape(128, 4, 256)
    ).astype(np.float16)
    bias2 = np.ascontiguousarray(b_out.reshape(2, 128).T)

    in_maps = []
    for h in range(NCORES):
        # [c, 128] per half with columns [q 64 | k 64] stacked -> M=128
        wqk = np.concatenate([wq[h].T, wk[h].T], axis=1)  # [256, 128]
        wqk_packed = np.ascontiguousarray(
            wqk.reshape(2, 128, 128).transpose(1, 0, 2)).astype(np.float16)
        wv_packed = np.ascontiguousarray(
            wv[h].T.reshape(2, 128, 64).transpose(1, 0, 2).reshape(128, 128)
        ).astype(np.float16)
        in_maps.append({
            "nonce": np.zeros((1, _kernel_hash() % 509 + 3), np.float32),
            "x_t": x_in,
            "wqk_p": wqk_packed,
            "wv_p": wv_packed,
            "wo_p": wo_p,
            "bias2": bias2,
        })
    return in_maps


def _unshard(shards):
    # granule mapping: shard j's column s = ib*64 + t holds l = ib*512 +
    # j*64 + t
    y = shards.reshape(NCORES, B, C, 4, 64).transpose(1, 2, 3, 0, 4)
    return np.ascontiguousarray(y.reshape(B, C, L), np.float32)


def _run(inputs, trace=False):
    if "nc" not in _CACHE:
        _CACHE["nc"] = _build()
    nc = _CACHE["nc"]
    in_maps = _prep_in_maps(**inputs)
    res = bass_utils.run_bass_kernel_spmd(
        nc, in_maps, core_ids=list(range(NCORES)), trace=trace)
    shards = np.stack([res.results[j]["out"].reshape(B, C, LSH)
                       for j in range(NCORES)])
    return _unshard(shards), res


def kernel(x, w_qkv, w_out, b_out):
    y, _ = _run(dict(x=x, w_qkv=w_qkv, w_out=w_out, b_out=b_out), trace=False)
    return y


# revision 34
# speedup vs baseline: 1.1070x; 1.1070x over previous
"""Trainium2 Bass kernel for nn_Attention (B=4, C=256, L=2048, H=8 heads, D=64).

Sharding: head-parallel across 8 NeuronCores (1 head per core). Each core:
  - projects its head's Q/K/V from the full input x (channels-first, fp16),
  - runs attention in the S^T (keys-on-partitions) layout so softmax's
    denominator comes free from an appended ones-column in the V^T lhsT
    (M=65 matmul),
  - normalizes + casts its head output to fp16,
  - per half-batch AllToAlls redistribute head outputs: each core k owns
    64-column granules g with g % 8 == k of every batch (granule mapping:
    l = ib*512 + k*64 + t for shard column s = ib*64 + t),
  - each core applies w_out + bias on its column shard.
Host reassembles the 8 granule-interleaved shards into the full [B, C, L]
output.

The attention phase is ACT(Exp)-bound (~1.1us per (ib,jp) step); the whole
(b, ib, jp) sequence is software-pipelined as one stream with the PV matmul
running two steps behind S, and the PE's leftover per-step time is filled
with the next batch's QKV projection and the previous batch's output
projection, one small chunk per slot, so the Exp stream never starves.
"""

import os
import sys
from collections import deque

import numpy as np

sys.path.insert(0, "/opt/trn_rl_repo")

import concourse.bass as bass  # noqa: E402
import concourse.bacc as bacc  # noqa: E402
import concourse.tile as tile  # noqa: E402
import concourse.mybir as mybir  # noqa: E402
import concourse.bass_utils as bass_utils  # noqa: E402
from concourse.bass_interp import get_hw_module  # noqa: E402

B, C, L = 4, 256, 2048
H, D = 8, 64
NCORES = 8
N = B * L                # 8192 flattened (b, l) columns
LSH = L // NCORES        # 256 l-columns per core in the output shard
NBLK = 512               # matmul free-dim block
F32 = mybir.dt.float32
F16 = mybir.dt.float16
AF = mybir.ActivationFunctionType

_CACHE = {}


def _kernel_hash():
    import hashlib
    with open(os.path.abspath(__file__), 'rb') as f:
        return int(hashlib.sha256(f.read()).hexdigest(), 16)


def _build():
    nc = bacc.Bacc("TRN2", target_bir_lowering=False, debug=False,
                   num_devices=NCORES)

    # The executable cache upstream keys on the HLO signature, which does not
    # cover the embedded BIR — encode a hash of this file in a dummy input's
    # shape so different kernel builds can't collide.
    nonce_len = _kernel_hash() % 509 + 3
    nc.dram_tensor("nonce", [1, nonce_len], F32, kind="ExternalInput")

    x_t = nc.dram_tensor("x_t", [2, 128, N], F16, kind="ExternalInput")
    # [c_lo, ch, (q|k) out] merged Q+K projection weights
    wqk_p = nc.dram_tensor("wqk_p", [128, 2, 128], F16, kind="ExternalInput")
    wv_p = nc.dram_tensor("wv_p", [128, 128], F16, kind="ExternalInput")
    wo_p = nc.dram_tensor("wo_p", [128, 4, 256], F16, kind="ExternalInput")
    bias2 = nc.dram_tensor("bias2", [128, 2], F32, kind="ExternalInput")
    out = nc.dram_tensor("out", [B, 2, 128, LSH], F32, kind="ExternalOutput")

    with tile.TileContext(nc) as tc:
        with (
            tc.tile_pool(name="const", bufs=1) as cpool,
            tc.tile_pool(name="qk", bufs=3) as qkpool,
            tc.tile_pool(name="vt", bufs=3) as vtpool,
            tc.tile_pool(name="pt", bufs=4) as ptpool,
            tc.tile_pool(name="small", bufs=4) as spool,
            tc.tile_pool(name="gh", bufs=2) as ghpool,
            tc.tile_pool(name="psA", bufs=2, space="PSUM") as psA,
            tc.tile_pool(name="psO", bufs=2, space="PSUM") as psO,
            tc.tile_pool(name="psP", bufs=2, space="PSUM") as psP,
            tc.tile_pool(name="dram", bufs=1, space="DRAM") as dpool,
        ):
            # ---- weights + batch-0 x first (fine chunks so the batch-0
            # projection can start as soon as each 512-col block lands) ----
            wqk_sb = cpool.tile([128, 256], F16, name="wqk_sb")
            wv_sb = cpool.tile([128, 128], F16, name="wv_sb")
            wo_sb = cpool.tile([128, 1024], F16, name="wo_sb")
            bias_sb = cpool.tile([128, 2], F32, name="bias_sb")
            nc.sync.dma_start(wqk_sb.rearrange("p (c o) -> p c o", c=2), wqk_p[:])
            nc.sync.dma_start(wv_sb[:], wv_p[:])
            x_sb = cpool.tile([128, 2 * N], F16, name="x_sb")

            # tiny warmup AllToAll: absorbs communicator init + first-use
            # slowness while the x DMA streams, so the first real a2a runs
            # at steady-state speed
            warm_sb = cpool.tile([NCORES, 2], F16, name="warm_sb")
            nc.vector.memset(warm_sb[:], 0.0)
            warm_in = dpool.tile([NCORES, 1, 2], F16, name="warm_in",
                                 tag="warm_in")
            warm_out = dpool.tile([NCORES, 1, 2], F16, name="warm_out",
                                  tag="warm_out")
            nc.sync.dma_start(warm_in[:, 0, :], warm_sb[:])
            nc.gpsimd.collective_compute(
                "AllToAll", mybir.AluOpType.bypass,
                replica_groups=[list(range(NCORES))],
                ins=[warm_in.opt()], outs=[warm_out.opt()])

            def dma_x(c0, c1):
                for ch in range(2):
                    nc.sync.dma_start(
                        x_sb[:, ch * N + c0:ch * N + c1],
                        x_t[ch, :, c0:c1])

            for blk in range(4):                     # batch 0, 512-col blocks
                dma_x(blk * NBLK, (blk + 1) * NBLK)
            nc.sync.dma_start(wo_sb.rearrange("p (c o) -> p c o", c=4), wo_p[:])
            nc.sync.dma_start(bias_sb[:], bias2[:])
            for s in range(2, 8):                    # batches 1-3
                dma_x(s * 1024, (s + 1) * 1024)

            # a2a staging: batches 0-2 use one whole-batch AllToAll;
            # the last batch is split into two half-batch AllToAlls (with
            # separate tiles so the Tile dependency tracker can't couple the
            # halves) to shrink the end-of-kernel tail.
            def _mk_bnc(pfx):
                ts = [dpool.tile([NCORES, 64, 256], F16, name=f"{pfx}{b}",
                                 tag=f"{pfx}{b}") for b in range(B - 1)]
                last = [dpool.tile([NCORES, 64, 128], F16, name=f"{pfx}3{h}",
                                   tag=f"{pfx}3{h}") for h in range(2)]
                return ts, last
            bnc_in, bnc_in3 = _mk_bnc("bi")
            bnc_out, bnc_out3 = _mk_bnc("bo")

            qd = {}
            kd = {}
            vt3 = {}

            # ---------------- filler emission (PE keep-warm) ----------------
            # Each filler is tagged 'pe' (matmul work) or 'vec' (vector/DMA
            # work). 'vec' fillers are only popped in mid-ib slots so they
            # can't queue ahead of the per-ib recip/mul normalization chain
            # on the vector engine (which feeds the a2a trigger).
            fillers = deque()

            def filler(vec_ok=True):
                # pop the first item allowed in this slot whose python-level
                # prerequisites have been emitted (scan a small window so a
                # blocked head can't starve the slot)
                for idx in range(min(len(fillers), 8)):
                    kind, ready, fn = fillers[idx][:3]
                    if (vec_ok or kind == 'pe') and (ready is None or ready()):
                        del fillers[idx]
                        fn()
                        return

            def drain_tagged(tag):
                # Tile tracks data deps in emission order, so everything a
                # batch consumes must be emitted before its first consumer:
                # force-drain the tagged package at its deadline.
                rest = deque()
                while fillers:
                    item = fillers.popleft()
                    if item[3] == tag:
                        item[2]()
                    else:
                        rest.append(item)
                fillers.extend(rest)

            def drain_fillers():
                while fillers:
                    fillers.popleft()[2]()

            def emit_projvt_blk(b, blk, st):
                """QK proj + V^T for one 512-col block of batch b (direct
                emission; used for batch 0 under the initial x DMA)."""
                emit_qk_mm(b, blk, 0, st)
                emit_qk_mm(b, blk, 1, st)
                emit_qk_copy(b, blk, st)
                for jp in range(2 * blk, 2 * blk + 2):
                    emit_vt_mm(b, jp, 0, st)
                    emit_vt_mm(b, jp, 1, st)
                    emit_vt_copy(b, jp, st)

            def emit_alloc(b):
                qd[b] = qkpool.tile([128, L], F16, name="qd", tag="qd")
                kd[b] = qkpool.tile([128, L], F16, name="kd", tag="kd")
                vt3[b] = vtpool.tile(
                    [128, 16 * 65], F16, name="vt", tag="vt"
                ).rearrange("p (j e) -> p j e", e=65)
                nc.vector.memset(vt3[b][:, :, 64], 1.0)

            def emit_qk_mm(b, nb, ch, st):
                if ch == 0:
                    st[('ps', nb)] = psP.tile([128, NBLK], F32, name="psqk",
                                              tag="psp")
                col0 = ch * N + b * L + nb * NBLK
                nc.tensor.matmul(
                    st[('ps', nb)][:], wqk_sb[:, ch * 128:(ch + 1) * 128],
                    x_sb[:, col0:col0 + NBLK],
                    start=(ch == 0), stop=(ch == 1))

            def emit_qk_copy(b, nb, st):
                ps = st.pop(('ps', nb))
                nc.vector.tensor_copy(
                    qd[b][0:64, nb * NBLK:(nb + 1) * NBLK], ps[0:64, :])
                nc.vector.tensor_copy(
                    kd[b][0:64, nb * NBLK:(nb + 1) * NBLK], ps[64:128, :])
                nc.vector.tensor_copy(
                    qd[b][64:128, nb * NBLK:(nb + 1) * NBLK],
                    qd[b][0:64, nb * NBLK:(nb + 1) * NBLK])
                nc.vector.tensor_copy(
                    kd[b][64:128, nb * NBLK:(nb + 1) * NBLK],
                    kd[b][0:64, nb * NBLK:(nb + 1) * NBLK])

            def emit_vt_mm(b, jp, half, st):
                # V^T directly: out[l, d] = sum_c x[c, l] * wv[c, d]
                if half == 0:
                    st[('pst', jp)] = psP.tile([128, 128], F32, name="pst",
                                               tag="psp")
                jt = 2 * jp + half
                for ch in range(2):
                    col0 = ch * N + b * L + jt * 128
                    nc.tensor.matmul(
                        st[('pst', jp)][:, half * 64:(half + 1) * 64],
                        x_sb[:, col0:col0 + 128],
                        wv_sb[:, ch * 64:(ch + 1) * 64],
                        start=(ch == 0), stop=(ch == 1))

            def emit_vt_copy(b, jp, st):
                nc.vector.tensor_copy(
                    vt3[b][:, 2 * jp:2 * jp + 2, 0:64],
                    st.pop(('pst', jp)).rearrange("p (j e) -> p j e", e=64))

            def queue_projvt(b):
                st = {}
                emit_alloc(b)
                tag = ('proj', b)
                for nb in range(4):
                    fillers.append(('pe', None,
                                    lambda nb=nb: emit_qk_mm(b, nb, 0, st), tag))
                    fillers.append(('pe', None,
                                    lambda nb=nb: emit_qk_mm(b, nb, 1, st), tag))
                    fillers.append(('vec',
                                    (lambda nb=nb: ('ps', nb) in st),
                                    lambda nb=nb: emit_qk_copy(b, nb, st), tag))
                for jp in range(8):
                    fillers.append(('pe', None,
                                    lambda jp=jp: emit_vt_mm(b, jp, 0, st), tag))
                    fillers.append(('pe', None,
                                    lambda jp=jp: emit_vt_mm(b, jp, 1, st), tag))
                    fillers.append(('vec',
                                    (lambda jp=jp: ('pst', jp) in st),
                                    lambda jp=jp: emit_vt_copy(b, jp, st), tag))

            def queue_yproj(b, h):
                """Output projection of batch b, half h (after the (b, h)
                a2a)."""
                st = {}

                def gather():
                    st['gh'] = ghpool.tile([128, 4, 128], F16, name="gh",
                                           tag=f"gh{h}")
                    for hc in range(4):
                        for hp in range(2):
                            src_ap = (bnc_out3[h][hc * 2 + hp, :, :]
                                      if b == B - 1 else
                                      bnc_out[b][hc * 2 + hp, :,
                                                 h * 128:(h + 1) * 128])
                            nc.sync.dma_start(
                                st['gh'][hp * 64:(hp + 1) * 64, hc, :], src_ap)

                tag = ('yproj', b, h)
                fillers.append(('vec', None, gather, tag))

                def y_mm(oh, cpair):
                    if cpair == 0:
                        st[('psy', oh)] = psP.tile([128, 128], F32, name="psy",
                                                   tag="psp")
                    for c in (2 * cpair, 2 * cpair + 1):
                        nc.tensor.matmul(
                            st[('psy', oh)][:],
                            wo_sb[:, c * 256 + oh * 128:c * 256 + (oh + 1) * 128],
                            st['gh'][:, c, :],
                            start=(c == 0), stop=(c == 3))

                def y_out(oh):
                    y = spool.tile([128, 128], F32, name="y", tag="y")
                    nc.vector.tensor_scalar_add(y[:], st.pop(('psy', oh)),
                                                bias_sb[:, oh:oh + 1])
                    nc.sync.dma_start(out[b, oh, :, h * 128:(h + 1) * 128], y[:])

                for oh in range(2):
                    fillers.append(('pe', (lambda: 'gh' in st),
                                    lambda oh=oh: y_mm(oh, 0), tag))
                    fillers.append(('pe', (lambda oh=oh: ('psy', oh) in st),
                                    lambda oh=oh: y_mm(oh, 1), tag))
                    fillers.append(('vec', (lambda oh=oh: ('psy', oh) in st),
                                    lambda oh=oh: y_out(oh), tag))

            # ---------------- attention stream ----------------
            psos = {}
            pts = {}

            def emit_s(b, ib, jp):
                jA, jB = 2 * jp, 2 * jp + 1
                pss = psA.tile([128, 2 * NBLK], F32, name="pss", tag="pss")
                nc.tensor.matmul(
                    pss[:, 0:NBLK],
                    kd[b][0:64, jA * 128:(jA + 1) * 128],
                    qd[b][0:64, ib * NBLK:(ib + 1) * NBLK],
                    start=True, stop=True, tile_position=(0, 0))
                nc.tensor.matmul(
                    pss[:, NBLK:2 * NBLK],
                    kd[b][64:128, jB * 128:(jB + 1) * 128],
                    qd[b][64:128, ib * NBLK:(ib + 1) * NBLK],
                    start=True, stop=True, tile_position=(64, 0))
                pt = ptpool.tile([128, 2 * NBLK], F16, name="pt", tag="pt")
                nc.scalar.activation(pt[:], pss[:], AF.Exp)
                pts[(b, ib, jp)] = pt

            def emit_pv(b, ib, jp):
                jA, jB = 2 * jp, 2 * jp + 1
                if jp == 0:
                    psos[(b, ib)] = psO.tile([65, NBLK], F32, name="pso",
                                             tag="pso")
                pso = psos[(b, ib)]
                pt = pts.pop((b, ib, jp))
                nc.tensor.matmul(
                    pso[:], vt3[b][:, jA, :], pt[:, 0:NBLK],
                    start=(jp == 0), stop=False)
                nc.tensor.matmul(
                    pso[:], vt3[b][:, jB, :], pt[:, NBLK:2 * NBLK],
                    start=False, stop=(jp == 7))
                if jp == 7:
                    emit_norm_out(b, ib)

            def emit_norm_out(b, ib):
                """Normalize + stage ib's output columns for the a2a; then
                trigger the half-batch a2a when its data is complete."""
                pso = psos.pop((b, ib))
                # custom-DVE ops need a base-partition-0 input: stage the
                # denominator row into SBUF first
                den = spool.tile([1, NBLK], F32, name="den", tag="den")
                nc.vector.tensor_copy(den[:], pso[64:65, :])
                recip = spool.tile([1, NBLK], F32, name="recip", tag="recip")
                nc.vector.reciprocal_approx_fast(recip[:], den[:])
                bc = spool.tile([64, NBLK], F32, name="bc", tag="bc")
                nc.gpsimd.partition_broadcast(bc[:], recip[:])
                on = spool.tile([64, NBLK], F16, name="on", tag="on")
                nc.vector.tensor_mul(on[:], pso[0:64, :], bc[:])
                # granule-interleaved shard mapping: column k*64+t of `on`
                # goes to core k's shard slot ib*64+t
                if b == B - 1:
                    dst = bnc_in3[ib // 2][:, :, (ib % 2) * 64:(ib % 2) * 64 + 64]
                else:
                    dst = bnc_in[b][:, :, ib * 64:(ib + 1) * 64]
                nc.sync.dma_start(dst.rearrange("k p t -> p k t"),
                                  on.rearrange("p (k t) -> p k t", k=8))
                grp = [list(range(NCORES))]
                if b == B - 1 and ib % 2 == 1:
                    nc.gpsimd.collective_compute(
                        "AllToAll", mybir.AluOpType.bypass, replica_groups=grp,
                        ins=[bnc_in3[ib // 2].opt()],
                        outs=[bnc_out3[ib // 2].opt()])
                elif b < B - 1 and ib == 3:
                    nc.gpsimd.collective_compute(
                        "AllToAll", mybir.AluOpType.bypass, replica_groups=grp,
                        ins=[bnc_in[b].opt()], outs=[bnc_out[b].opt()])

            # ---------------- schedule ----------------
            # Batch 0's projection is emitted block-by-block just ahead of
            # the attention steps that consume it, so the first Exp can
            # start as soon as the first 512 columns of x have landed.
            st0 = {}
            emit_alloc(0)
            for blk in range(4):
                emit_projvt_blk(0, blk, st0)

            steps = [(b, ib, jp)
                     for b in range(B) for ib in range(4) for jp in range(8)]
            pending = deque()
            for (b, ib, jp) in steps:
                if (ib, jp) == (0, 0) and b > 0:
                    drain_tagged(('proj', b))
                if (ib, jp) == (2, 0):
                    if b + 1 < B:
                        queue_projvt(b + 1)
                    if b > 0:
                        queue_yproj(b - 1, 0)
                if (ib, jp) == (3, 0) and b > 0:
                    queue_yproj(b - 1, 1)
                emit_s(b, ib, jp)
                filler()
                pending.append((b, ib, jp))
                if len(pending) > 2:
                    emit_pv(*pending.popleft())
                    filler()
            while pending:
                emit_pv(*pending.popleft())
            queue_yproj(B - 1, 0)
            queue_yproj(B - 1, 1)
            drain_fillers()

    nc.compile()
    if not os.environ.get("BASS_SIM"):
        nc.m = get_hw_module(nc.m)
    return nc


def _prep_in_maps(x, w_qkv, w_out, b_out):
    scale = float(D) ** -0.5
    x = np.asarray(x, np.float32)
    w_qkv = np.asarray(w_qkv, np.float32)
    w_out = np.asarray(w_out, np.float32)
    b_out = np.asarray(b_out, np.float32)

    x_in = np.ascontiguousarray(
        x.transpose(1, 0, 2).reshape(C, N).reshape(2, 128, N)).astype(np.float16)
    wq = w_qkv[0:512].reshape(H, D, C) * scale
    wk = w_qkv[512:1024].reshape(H, D, C)
    wv = w_qkv[1024:1536].reshape(H, D, C)

    wo_p = np.ascontiguousarray(
        w_out.T.reshape(4, 2, 64, 256).transpose(1, 2, 0, 3).reshape(128, 4, 256)
    ).astype(np.float16)
    bias2 = np.ascontiguousarray(b_out.reshape(2, 128).T)

    in_maps = []
    for h in range(NCORES):
        # [c, 128] per half with columns [q 64 | k 64] stacked -> M=128
        wqk = np.concatenate([wq[h].T, wk[h].T], axis=1)  # [256, 128]
        wqk_packed = np.ascontiguousarray(
            wqk.reshape(2, 128, 128).transpose(1, 0, 2)).astype(np.float16)
        wv_packed = np.ascontiguousarray(
            wv[h].T.reshape(2, 128, 64).transpose(1, 0, 2).reshape(128, 128)
        ).astype(np.float16)
        in_maps.append({
            "nonce": np.zeros((1, _kernel_hash() % 509 + 3), np.float32),
            "x_t": x_in,
            "wqk_p": wqk_packed,
            "wv_p": wv_packed,
            "wo_p": wo_p,
            "bias2": bias2,
        })
    return in_maps


def _unshard(shards):
    # granule mapping: shard j's column s = ib*64 + t holds l = ib*512 +
    # j*64 + t
    y = shards.reshape(NCORES, B, C, 4, 64).transpose(1, 2, 3, 0, 4)
    return np.ascontiguousarray(y.reshape(B, C, L), np.float32)


def _run(inputs, trace=False):
    if "nc" not in _CACHE:
        _CACHE["nc"] = _build()
    nc = _CACHE["nc"]
    in_maps = _prep_in_maps(**inputs)
    res = bass_utils.run_bass_kernel_spmd(
        nc, in_maps, core_ids=list(range(NCORES)), trace=trace)
    shards = np.stack([res.results[j]["out"].reshape(B, C, LSH)
                       for j in range(NCORES)])
    return _unshard(shards), res


def kernel(x, w_qkv, w_out, b_out):
    y, _ = _run(dict(x=x, w_qkv=w_qkv, w_out=w_out, b_out=b_out), trace=False)
    return y
